# revision 39
# baseline (speedup 1.0000x reference)
"""Dilated block attention + output projection on 8 trn2 cores.

Sharding: core c handles batch b = c//2 and heads h = 4*(c%2) .. +3.
Each core computes the full dilated-attention combine for its 4 (b,h)
pairs and a partial output projection (contraction over its 4 heads'
256 hidden dims).  The host sums the two half-hidden partials per batch
and adds the bias.

Math note: the reference's stabilized-softmax + detached-expsum
reweighting collapses to the unstabilized form
    out[p] = (sum_d exp(S_d) @ V_d  scattered to p) / (sum_d rowsum exp(S_d))
which is what the kernel computes (scores ~ N(0,1), no overflow risk).

v2 structure (from v1 trace analysis: steady state was ACT(exp)-paced at
~1250ns/job with PE waiting on exp, plus a 56us serial o_proj tail):
  - TWO (b,h) chains interleaved so the scalar engine never drains: while
    chain A's exp waits on its QK matmul, chain B's exp (ready) runs.
  - exp groups of 3 k-tiles (1536-col ACT instructions) amortize the
    ~400ns per-instruction ACT overhead.  PSUM: st [128,3,512]x2 bufs
    (6 banks) + pv [128,512]x2 (2 banks) = 8 banks exactly.
  - QK matmuls run as K=64 row-tile pairs (parity-split K across SBUF
    partition halves -> tile_position rows 0/64 execute concurrently).
  - o_proj contracts K=128 by stacking head pairs (odd head's normalized
    output is DMA-stacked onto partitions 64-127), halving matmul count
    and keeping the PE warm right after the last attention window.
  - per-512-window normalize: w is broadcast to 64 partitions with a
    zero-padded fp32r ones-matmul on row-tile T8 (same (64,128) PE mode
    as QK -> no tiling-mode drain), then reciprocal_approx_fast + mul on
    [64,512] shapes (v1 did a 4.3us single-partition reciprocal).
  - branch blobs prefetched across pairs (b0 triple-buffered), Q/K and V
    parts land via separate DMAs so QK can start before V arrives.

Device layout per (b,h), per dilation branch (unchanged from v1): one
blob [128, W] per branch holding Q^T duplicated onto both partition
halves, K^T k-tiles parity-split (even tiles on partitions 0-63, odd on
64-127), and V k-tile slabs [128, 65] with a ones column (PV matmul with
M=65 gives the exp row-sum on psum row 64 for free).
"""

import ml_dtypes
import numpy as np

BF16_NP = ml_dtypes.bfloat16

# exp(x/8) = p(x/64)^8, p = degree-4 minimax with p(0)=1 (coefs folded by
# 64^-k).  Rel err <= 1.2e-3 over |x| <= 48 (6 sigma of the N(0,64) scores).
# Runs as two fused custom-DVE ops so ~1/6 of the exp load comes off the
# scalar engine (the steady-state pacer).
EXPC = (
    0.01561601459980011,
    0.0001222426217282191,
    6.561347731803835e-07,
    2.4336668236202286e-09,
)
# The DVE exp path works (absmax/scale 9.4e-3 vs 6.0e-3 all-scalar) but gave
# no measured speedup: the wall time is power-throttle-bound (~72us of
# hw throttling per run), so shifting exp work between engines doesn't move
# the wall.  Disabled to keep the larger accuracy margin.
DVE_EXP = False


def _register_exp_ops():
    """Register the two custom DVE ops (idempotent, per-process)."""
    import re

    import concourse.dve_ops as DO
    from concourse.dve_spec import (
        C0,
        C1,
        C2,
        C3,
        One,
        Spec,
        Src0,
        _spill_c3_to_src1,
        sq,
    )

    if "EXP_P4_ANT" in DO._SUB_OPCODE_FOR_NAME:
        byname = {op.name: op for op in DO.OPS}
        return byname["EXP_P4_ANT"], byname["POW8_ANT"]

    body4 = _spill_c3_to_src1(
        One + Src0 * (C0 + Src0 * (C1 + Src0 * (C2 + Src0 * C3)))
    )

    def ref4(in0, in1, s0, s1, imm2):
        x = in0.astype(np.float32)
        return (1.0 + x * (s0 + x * (s1 + x * (imm2 + x * in1)))).astype(
            np.float32
        )

    e4 = DO.DveOp(
        "EXP_P4_ANT",
        Spec(body=body4, reference=ref4),
        subdim=False,
        uops_sha={"v3": "b79b87021d1db5c9", "v4": "7b41b728fe12a7dc"},
    )
    p8 = DO.DveOp(
        "POW8_ANT",
        Spec(
            body=sq(sq(sq(Src0))),
            reference=lambda in0, in1, s0, s1, imm2: (
                (in0.astype(np.float32) ** 2) ** 2
            )
            ** 2,
        ),
        subdim=False,
        uops_sha={"v3": "e38569d65e263694", "v4": "9690799ca8951127"},
    )
    for op in (e4, p8):
        DO.OPS.append(op)
        DO.CUSTOM_DVE_SPECS[op.name] = op.spec
        DO._SUB_OPCODE_FOR_NAME[op.name] = DO._CUSTOM_DVE_ROW_BASE + len(DO.OPS) - 1
        for ver in ("v3", "v4"):
            try:
                op.compile(ver)
            except ValueError as e:
                m = re.search(r'uops_sha\["(v\d)"\]="([0-9a-f]+)"', str(e))
                if not m:
                    raise
                op.uops_sha[m.group(1)] = m.group(2)
                op.compile(ver)
    return e4, p8

B, H, L, HD = 4, 8, 4096, 64
HIDDEN = H * HD
DILS = (1, 2, 4, 8)
BLOCK = 1024
PB = 4  # (b,h) pairs per core
NCORES = 8
LDS = [L // d for d in DILS]  # 4096, 2048, 1024, 512
NKTS = [ld // 128 for ld in LDS]  # 32, 16, 8, 4
# blob widths per branch: Q dup (Ld) + K parity-split (Ld/2) + V slabs (nkt*65)
WS = [ld + ld // 2 + nkt * 65 for ld, nkt in zip(LDS, NKTS)]
BOFFS = [sum(WS[:i]) for i in range(len(WS))]
WSUM = sum(WS)
QCH = 512  # q-window width (strided-domain positions)

_PROGRAM = None


def _build_jobs():
    """Job list for one chain: one job per exp-group (<=3 k-tiles of one
    512-q window)."""
    jobs = []
    for di, d in enumerate(DILS):
        Ld = LDS[di]
        bs = min(BLOCK, Ld)
        nblk = Ld // bs
        nkt_blk = bs // 128
        groups = [[0, 1, 2], [3, 4, 5], [6, 7]] if nkt_blk == 8 else [[0, 1], [2, 3]]
        for blk in range(nblk):
            for qc in range(bs // QCH):
                q0 = blk * bs + qc * QCH
                for gi, g in enumerate(groups):
                    jobs.append(
                        dict(
                            di=di,
                            d=d,
                            blk=blk,
                            nkt_blk=nkt_blk,
                            q0=q0,
                            g=g,
                            first=(gi == 0),
                            last=(gi == len(groups) - 1),
                            done0=g[0],
                        )
                    )
    return jobs


def build_program():
    """Build the (SPMD, identical on all cores) Bass program."""
    from contextlib import ExitStack

    import concourse.tile as tile
    from concourse import bacc, mybir

    F32 = mybir.dt.float32
    BF16 = mybir.dt.bfloat16
    EXP_P4, POW8 = _register_exp_ops() if DVE_EXP else (None, None)
    nc = bacc.Bacc("TRN2", target_bir_lowering=False, debug=False)

    blob_d = nc.dram_tensor("blob", [PB, 128, WSUM], BF16, kind="ExternalInput")
    wot_d = nc.dram_tensor("wot", [2, 128, HIDDEN], BF16, kind="ExternalInput")
    out_d = nc.dram_tensor("out", [L, HIDDEN], F32, kind="ExternalOutput")

    with tile.TileContext(nc) as tc, ExitStack() as ctx:
        consts = ctx.enter_context(tc.tile_pool(name="consts", bufs=1))
        br_pool = ctx.enter_context(tc.tile_pool(name="br", bufs=2))
        e_pool = ctx.enter_context(tc.tile_pool(name="ep", bufs=4))
        acc_pool = ctx.enter_context(tc.tile_pool(name="accp", bufs=1))
        io_pool = ctx.enter_context(tc.tile_pool(name="iop", bufs=2))
        st_psum = ctx.enter_context(tc.tile_pool(name="stp", bufs=2, space="PSUM"))
        pv_psum = ctx.enter_context(tc.tile_pool(name="pvp", bufs=2, space="PSUM"))

        zero_bias = consts.tile([128, 1], F32, tag="zb")
        nc.vector.memset(zero_bias, 0.0)
        c4_ap = consts.tile([128, 1], F32, tag="c4")
        nc.vector.memset(c4_ap, EXPC[3])
        # bf16 ones row at partition 64, zeros on 65..127: K=64 zero-padded
        # broadcast weights so the w-broadcast matmul shares the QK matmuls'
        # (64,128) row-tiled PE mode (T8) instead of forcing a mode drain.
        onespad = consts.tile([128, 128], BF16, tag="ones")
        nc.vector.memset(onespad, 0.0)
        nc.vector.memset(onespad[64:65, :], 1.0)
        # staging tiles for the bf16 w row at partition 64; rows 65..127 are
        # zeroed once and never rewritten (only row 64 is written per window)
        wb_tiles = [
            consts.tile([128, QCH], BF16, tag="wb", bufs=2, name=f"wb{i}")
            for i in range(2)
        ]
        for wb in wb_tiles:
            nc.vector.memset(wb[64:128, :], 0.0)
        wb_ctr = [0]

        wot_sb = consts.tile([128, 2, HIDDEN], BF16, tag="wot")
        wot_loaded = [False]

        def load_wot():
            # deferred past the first blob pieces so it doesn't delay the
            # cold-start QK matmuls
            if not wot_loaded[0]:
                wot_loaded[0] = True
                nc.sync.dma_start(
                    out=wot_sb, in_=wot_d.rearrange("j r c -> r j c")
                )

        acc_tiles = [
            acc_pool.tile([128, L], F32, tag=f"acc{j}", bufs=1, name=f"acc{j}")
            for j in range(PB)
        ]

        oacc_pairs = [
            acc_pool.tile([128, L], BF16, tag=f"oacc{p}", bufs=1, name=f"oacc{p}")
            for p in range(2)
        ]
        oacc_tmp = acc_pool.tile([64, L], BF16, tag="otmp", bufs=1, name="oacc_tmp")

        bt_tiles = {}
        dma_issued = set()

        def issue_blob(j, di, v_part=True):
            if j >= PB:
                return
            if (j, di) not in dma_issued:
                dma_issued.add((j, di))
                bufs = 3 if di == 0 else 2
                bt = br_pool.tile(
                    [128, WS[di]], BF16, tag=f"b{di}", bufs=bufs, name=f"bt{di}"
                )
                Ld = LDS[di]
                qk_w = Ld + Ld // 2
                if di == 0 and j < 2:
                    # cold start: land the first window's Q and K columns
                    # first so QK matmuls begin before the full blob arrives
                    pieces = ((0, 1024), (Ld, Ld + 512), (1024, Ld), (Ld + 512, qk_w))
                else:
                    pieces = ((0, qk_w),)
                for c0, c1 in pieces:
                    nc.sync.dma_start(
                        out=bt[:, c0:c1],
                        in_=blob_d[j, :, BOFFS[di] + c0 : BOFFS[di] + c1],
                    )
                bt_tiles[(j, di)] = bt
            if v_part and (j, di, "v") not in dma_issued:
                dma_issued.add((j, di, "v"))
                bt = bt_tiles[(j, di)]
                Ld = LDS[di]
                qk_w = Ld + Ld // 2
                nc.sync.dma_start(
                    out=bt[:, qk_w : WS[di]],
                    in_=blob_d[j, :, BOFFS[di] + qk_w : BOFFS[di] + WS[di]],
                )

        def emit_qk_exp(j, job):
            """QK matmuls for the group -> exp to a bf16 E tile."""
            di, q0, g = job["di"], job["q0"], job["g"]
            Ld = LDS[di]
            bt = bt_tiles[(j, di)]
            gs = len(g)
            st = st_psum.tile([128, 3, QCH], F32, tag="st", name="st")
            for i, kt in enumerate(g):
                tg = job["blk"] * job["nkt_blk"] + kt
                # K parity is flipped on odd pairs (host packs it that way) so
                # the QK stream alternates T0/T8 row tiles across both chains
                # — every matmul overlaps its neighbor on the other row tile.
                half = (tg + j) % 2
                k0 = Ld + (tg // 2) * 128
                nc.tensor.matmul(
                    st[:, i, :],
                    bt[half * 64 : (half + 1) * 64, k0 : k0 + 128],
                    bt[half * 64 : (half + 1) * 64, q0 : q0 + QCH],
                    start=True,
                    stop=True,
                )
            et = e_pool.tile([128, 3, QCH], BF16, tag="et", bufs=5, name="et")
            if job.get("dve"):
                ex = e_pool.tile([128, 3, QCH], F32, tag="ex", bufs=2, name="ex")
                nc.vector._custom_dve(
                    EXP_P4,
                    out=ex[:, 0:gs, :],
                    in0=st[:, 0:gs, :],
                    in1=c4_ap,
                    s0=EXPC[0],
                    s1=EXPC[1],
                    imm2=EXPC[2],
                )
                nc.vector._custom_dve(POW8, out=et[:, 0:gs, :], in0=ex[:, 0:gs, :])
            else:
                nc.scalar.activation(
                    et[:, 0:gs, :],
                    st[:, 0:gs, :],
                    mybir.ActivationFunctionType.Exp,
                    bias=zero_bias,
                    scale=0.125,
                )
            job["et"] = et

        def emit_pv_one(j, job, i):
            di = job["di"]
            Ld = LDS[di]
            vbase = Ld + Ld // 2
            bt = bt_tiles[(j, di)]
            kt = job["g"][i]
            tg = job["blk"] * job["nkt_blk"] + kt
            done = job["done0"] + i
            nc.tensor.matmul(
                job["pv"][0:65, :],
                bt[:, vbase + tg * 65 : vbase + tg * 65 + 65],
                job["et"][:, i, :],
                start=(done == 0),
                stop=(done == job["nkt_blk"] - 1),
                skip_group_check=True,
            )

        def emit_scatter(j, job):
            if not job["last"]:
                return
            d = job["d"]
            acc = acc_tiles[j]
            pv = job["pv"]
            p0 = job["q0"] * d
            if d == 1:
                # scalar engine has slack in the main phase; keeping this off
                # the DVE FIFO lets the strided scatters (which free pv slots)
                # run sooner
                nc.scalar.copy(out=acc[0:65, p0 : p0 + QCH], in_=pv[0:65, :])
            else:
                dst = acc[0:65, p0 : p0 + QCH * d : d]
                nc.vector.tensor_add(out=dst, in0=dst, in1=pv[0:65, :])

        def emit_pv_pair(job_a, job_b, j_a, j_b):
            """Both chains' PV groups, each chain's accumulation run kept
            contiguous (interleaving the two runs makes walrus break the
            accumulation groups and stalls the PE)."""
            for i in range(len(job_a["g"])):
                emit_pv_one(j_a, job_a, i)
            for i in range(len(job_b["g"])):
                emit_pv_one(j_b, job_b, i)
            emit_scatter(j_a, job_a)
            emit_scatter(j_b, job_b)

        def emit_norm(pp, w, j, tail=False):
            """Normalize one 512-wide window of pair j: stage the bf16 w row
            at partition 64, broadcast it to all partitions with a zero-padded
            K=64 ones-matmul on row-tile T8 (bc rides a spare slice of the st
            PSUM slot), invert on DVE, scale the numerator.  Odd pairs stage
            into oacc_tmp and DMA-stack onto partitions 64..127 for o_proj."""
            ws = slice(w * QCH, (w + 1) * QCH)
            acc = acc_tiles[j]
            wb = wb_tiles[wb_ctr[0] % 2]
            wb_ctr[0] += 1
            if tail:
                nc.scalar.copy(out=wb[64:65, :], in_=acc[64:65, ws])
            else:
                nc.vector.tensor_copy(out=wb[64:65, :], in_=acc[64:65, ws])
            bc = st_psum.tile([128, QCH], F32, tag="st", name="bc")
            nc.tensor.matmul(
                bc,
                onespad[64:128, :],
                wb[64:128, :],
                start=True,
                stop=True,
                tile_position=(64, 0),
            )
            rw = io_pool.tile([64, QCH], F32, tag="rw", name="rw")
            nc.vector.reciprocal_approx_fast(out=rw, in_=bc[0:64, :])
            odd = j % 2 == 1
            dst = oacc_tmp[:, ws] if odd else oacc_pairs[pp][0:64, ws]
            nc.vector.tensor_mul(out=dst, in0=acc[0:64, ws], in1=rw)
            if odd:
                nc.sync.dma_start(
                    out=oacc_pairs[pp][64:128, ws], in_=oacc_tmp[:, ws]
                )

        pending_norm = []
        for pp in range(2):
            ja, jb = 2 * pp, 2 * pp + 1
            issue_blob(ja, 0, v_part=False)
            issue_blob(jb, 0, v_part=False)
            issue_blob(ja, 0)
            issue_blob(jb, 0)
            issue_blob(ja, 1)
            issue_blob(jb, 1)
            load_wot()
            jobs = {ja: _build_jobs(), jb: _build_jobs()}
            njobs = len(jobs[ja])
            if DVE_EXP:
                # route a staggered ~1/6 of exp groups to the vector engine
                for g in range(njobs - 2):
                    if g % 6 == 1:
                        jobs[ja][g]["dve"] = True
                    elif g % 6 == 4:
                        jobs[jb][g]["dve"] = True
            prev_di = 0
            for g in range(njobs):
                di = jobs[ja][g]["di"]
                if di != prev_di:
                    prev_di = di
                    if di == 1:
                        issue_blob(ja, 2)
                        issue_blob(jb, 2)
                    elif di == 2:
                        issue_blob(ja, 3)
                        issue_blob(jb, 3)
                        issue_blob(ja + 2, 0)
                    elif di == 3:
                        issue_blob(jb + 2, 0)
                        issue_blob(ja + 2, 1)
                        issue_blob(jb + 2, 1)
                # batch both chains' QKs, then both chains' PVs: QK (64-row
                # tiled) and PV (128-row) use different PE tiling modes and
                # each mode change drains the array — 2 switches/iteration
                # instead of 4.
                for j in (ja, jb):
                    job = jobs[j][g]
                    if job["first"]:
                        job["pv"] = pv_psum.tile(
                            [128, QCH], F32, tag="pv", name="pv"
                        )
                    else:
                        job["pv"] = jobs[j][g - 1]["pv"]
                    emit_qk_exp(j, job)
                if g >= 2:
                    emit_pv_pair(jobs[ja][g - 2], jobs[jb][g - 2], ja, jb)
                # drip the previous pair-pair's normalize through this
                # pair-pair's job stream (it touches none of our tiles)
                if jobs[ja][g]["last"]:
                    for _ in range(2):
                        if pending_norm:
                            emit_norm(*pending_norm.pop(0))
            emit_pv_pair(jobs[ja][njobs - 2], jobs[jb][njobs - 2], ja, jb)
            emit_pv_pair(jobs[ja][njobs - 1], jobs[jb][njobs - 1], ja, jb)
            while pending_norm:
                emit_norm(*pending_norm.pop(0))
            pending_norm = [(pp, w, j) for w in range(L // QCH) for j in (ja, jb)]
            for j in (ja, jb):
                del bt_tiles[(j, 0)], bt_tiles[(j, 1)]
                del bt_tiles[(j, 2)], bt_tiles[(j, 3)]

        # tail: last pair-pair's normalize pipelined (one window ahead) with
        # the partial o_proj (out[p, :] = sum_pp oacc_pp[:, p]^T @ wot_pp,
        # K=128 per matmul).  PSUM-evacuation copies alternate between the
        # now-idle scalar engine and the vector engine.
        def emit_oproj(w):
            for mt in range(4 * w, 4 * w + 4):
                # alternate po between the pv slots and the (tail-idle) st
                # slots: 4 banks in flight so the matmul chain never waits on
                # the PSUM-evacuation copy two tiles back
                pool_tag = "pv" if mt % 2 == 0 else "st"
                src = pv_psum if mt % 2 == 0 else st_psum
                po = src.tile([128, HIDDEN], F32, tag=pool_tag, name="po")
                for p in range(2):
                    nc.tensor.matmul(
                        po,
                        oacc_pairs[p][:, mt * 128 : (mt + 1) * 128],
                        wot_sb[:, p, :],
                        start=(p == 0),
                        stop=(p == 1),
                        skip_group_check=True,
                    )
                ot = io_pool.tile([128, HIDDEN], F32, tag="ot", bufs=3)
                if mt % 2 == 0:
                    nc.scalar.copy(out=ot, in_=po)
                else:
                    nc.vector.tensor_copy(out=ot, in_=po)
                nc.sync.dma_start(out=out_d[mt * 128 : (mt + 1) * 128, :], in_=ot)

        for w in range(L // QCH):
            emit_norm(*pending_norm.pop(0), tail=True)
            emit_norm(*pending_norm.pop(0), tail=True)
            if w >= 1:
                emit_oproj(w - 1)
        emit_oproj(L // QCH - 1)

    nc.compile()
    return nc


def get_program():
    global _PROGRAM
    if _PROGRAM is None:
        _PROGRAM = build_program()
    return _PROGRAM


def _branch_blob(qT, kT, vv, di, flip):
    """Pack one dilation branch into the [128, W] SBUF-layout blob.

    qT, kT: [64, Ld] transposed Q/K for this branch; vv: [Ld, 65] V plus
    ones column.  flip swaps which partition half holds even k-tiles (odd
    pairs are flipped so the interleaved QK stream bricks across PE row
    tiles T0/T8)."""
    Ld, nkt = LDS[di], NKTS[di]
    q_part = np.concatenate([qT, qT], axis=0)  # [128, Ld]
    k3 = kT.reshape(64, nkt, 128)
    halves = [k3[:, 0::2, :].reshape(64, -1), k3[:, 1::2, :].reshape(64, -1)]
    if flip:
        halves = halves[::-1]
    k_part = np.concatenate(halves, axis=0)  # [128, Ld/2]
    v_part = vv.reshape(nkt, 128, 65).transpose(1, 0, 2).reshape(128, nkt * 65)
    return np.concatenate([q_part, k_part, v_part], axis=1)


def make_in_maps(query_states, key_states, value_states, Wo):
    q = np.asarray(query_states, dtype=np.float32)
    k = np.asarray(key_states, dtype=np.float32)
    v = np.asarray(value_states, dtype=np.float32)
    Wo = np.asarray(Wo, dtype=np.float32)

    in_maps = []
    for c in range(NCORES):
        b, hs = c // 2, (c % 2) * PB
        blob = np.empty((PB, 128, WSUM), BF16_NP)
        wot = np.empty((2, 128, HIDDEN), BF16_NP)
        for j in range(PB):
            h = hs + j
            for di, d in enumerate(DILS):
                Ld = LDS[di]
                vv = np.empty((Ld, 65), np.float32)
                vv[:, 0:64] = v[b, h, ::d, :]
                vv[:, 64] = 1.0
                blob[j, :, BOFFS[di] : BOFFS[di] + WS[di]] = _branch_blob(
                    np.ascontiguousarray(q[b, h, ::d, :].T),
                    np.ascontiguousarray(k[b, h, ::d, :].T),
                    vv,
                    di,
                    j % 2 == 1,
                )
        for p in range(2):
            h0 = hs + 2 * p
            wot[p] = Wo[:, h0 * 64 : (h0 + 2) * 64].T
        in_maps.append({"blob": blob, "wot": wot})
    return in_maps


def combine_outputs(results, bo):
    bo = np.asarray(bo, dtype=np.float32)
    out = np.empty((B, L, HIDDEN), np.float32)
    for b in range(B):
        out[b] = results[2 * b]["out"] + results[2 * b + 1]["out"] + bo
    return out


def kernel(
    query_states,
    key_states,
    value_states,
    Wo,
    bo,
    _trace=False,
    _tmpdir=None,
    _results=[None],
):
    from concourse.bass_utils import run_bass_kernel_spmd

    nc = get_program()
    in_maps = make_in_maps(query_states, key_states, value_states, Wo)
    res = run_bass_kernel_spmd(
        nc, in_maps, list(range(NCORES)), trace=_trace, tmpdir=_tmpdir
    )
    _results[0] = res
    return combine_outputs(res.results, bo)


# revision 42
# speedup vs baseline: 1.0641x; 1.0641x over previous
"""Dilated block attention + output projection on 8 trn2 cores.

Sharding: core c handles batch b = c//2 and heads h = 4*(c%2) .. +3.
Each core computes the full dilated-attention combine for its 4 (b,h)
pairs and a partial output projection (contraction over its 4 heads'
256 hidden dims).  The host sums the two half-hidden partials per batch
and adds the bias.

Math note: the reference's stabilized-softmax + detached-expsum
reweighting collapses to the unstabilized form
    out[p] = (sum_d exp(S_d) @ V_d  scattered to p) / (sum_d rowsum exp(S_d))
which is what the kernel computes (scores ~ N(0,1), no overflow risk).

v2 structure (from v1 trace analysis: steady state was ACT(exp)-paced at
~1250ns/job with PE waiting on exp, plus a 56us serial o_proj tail):
  - TWO (b,h) chains interleaved so the scalar engine never drains: while
    chain A's exp waits on its QK matmul, chain B's exp (ready) runs.
  - exp groups of 3 k-tiles (1536-col ACT instructions) amortize the
    ~400ns per-instruction ACT overhead.  PSUM: st [128,3,512]x2 bufs
    (6 banks) + pv [128,512]x2 (2 banks) = 8 banks exactly.
  - QK matmuls run as K=64 row-tile pairs (parity-split K across SBUF
    partition halves -> tile_position rows 0/64 execute concurrently).
  - o_proj contracts K=128 by stacking head pairs (odd head's normalized
    output is DMA-stacked onto partitions 64-127), halving matmul count
    and keeping the PE warm right after the last attention window.
  - per-512-window normalize: w is broadcast to 64 partitions with a
    zero-padded fp32r ones-matmul on row-tile T8 (same (64,128) PE mode
    as QK -> no tiling-mode drain), then reciprocal_approx_fast + mul on
    [64,512] shapes (v1 did a 4.3us single-partition reciprocal).
  - branch blobs prefetched across pairs (b0 triple-buffered), Q/K and V
    parts land via separate DMAs so QK can start before V arrives.

Device layout per (b,h), per dilation branch (unchanged from v1): one
blob [128, W] per branch holding Q^T duplicated onto both partition
halves, K^T k-tiles parity-split (even tiles on partitions 0-63, odd on
64-127), and V k-tile slabs [128, 65] with a ones column (PV matmul with
M=65 gives the exp row-sum on psum row 64 for free).
"""

import ml_dtypes
import numpy as np

BF16_NP = ml_dtypes.bfloat16

# exp(x/8) = p(x/64)^8, p = degree-4 minimax with p(0)=1 (coefs folded by
# 64^-k).  Rel err <= 1.2e-3 over |x| <= 48 (6 sigma of the N(0,64) scores).
# Runs as two fused custom-DVE ops so ~1/6 of the exp load comes off the
# scalar engine (the steady-state pacer).
EXPC = (
    0.01561601459980011,
    0.0001222426217282191,
    6.561347731803835e-07,
    2.4336668236202286e-09,
)
# The DVE exp path works (absmax/scale 9.4e-3 vs 6.0e-3 all-scalar) but gave
# no measured speedup: the wall time is power-throttle-bound (~72us of
# hw throttling per run), so shifting exp work between engines doesn't move
# the wall.  Disabled to keep the larger accuracy margin.
DVE_EXP = False


def _register_exp_ops():
    """Register the two custom DVE ops (idempotent, per-process)."""
    import re

    import concourse.dve_ops as DO
    from concourse.dve_spec import (
        C0,
        C1,
        C2,
        C3,
        One,
        Spec,
        Src0,
        _spill_c3_to_src1,
        sq,
    )

    if "EXP_P4_ANT" in DO._SUB_OPCODE_FOR_NAME:
        byname = {op.name: op for op in DO.OPS}
        return byname["EXP_P4_ANT"], byname["POW8_ANT"]

    body4 = _spill_c3_to_src1(
        One + Src0 * (C0 + Src0 * (C1 + Src0 * (C2 + Src0 * C3)))
    )

    def ref4(in0, in1, s0, s1, imm2):
        x = in0.astype(np.float32)
        return (1.0 + x * (s0 + x * (s1 + x * (imm2 + x * in1)))).astype(
            np.float32
        )

    e4 = DO.DveOp(
        "EXP_P4_ANT",
        Spec(body=body4, reference=ref4),
        subdim=False,
        uops_sha={"v3": "b79b87021d1db5c9", "v4": "7b41b728fe12a7dc"},
    )
    p8 = DO.DveOp(
        "POW8_ANT",
        Spec(
            body=sq(sq(sq(Src0))),
            reference=lambda in0, in1, s0, s1, imm2: (
                (in0.astype(np.float32) ** 2) ** 2
            )
            ** 2,
        ),
        subdim=False,
        uops_sha={"v3": "e38569d65e263694", "v4": "9690799ca8951127"},
    )
    for op in (e4, p8):
        DO.OPS.append(op)
        DO.CUSTOM_DVE_SPECS[op.name] = op.spec
        DO._SUB_OPCODE_FOR_NAME[op.name] = DO._CUSTOM_DVE_ROW_BASE + len(DO.OPS) - 1
        for ver in ("v3", "v4"):
            try:
                op.compile(ver)
            except ValueError as e:
                m = re.search(r'uops_sha\["(v\d)"\]="([0-9a-f]+)"', str(e))
                if not m:
                    raise
                op.uops_sha[m.group(1)] = m.group(2)
                op.compile(ver)
    return e4, p8

B, H, L, HD = 4, 8, 4096, 64
HIDDEN = H * HD
DILS = (1, 2, 4, 8)
BLOCK = 1024
PB = 4  # (b,h) pairs per core
NCORES = 8
LDS = [L // d for d in DILS]  # 4096, 2048, 1024, 512
NKTS = [ld // 128 for ld in LDS]  # 32, 16, 8, 4
# blob widths per branch: Q dup (Ld) + K parity-split (Ld/2) + V slabs (nkt*65)
WS = [ld + ld // 2 + nkt * 65 for ld, nkt in zip(LDS, NKTS)]
BOFFS = [sum(WS[:i]) for i in range(len(WS))]
WSUM = sum(WS)
QCH = 512  # q-window width (strided-domain positions)

_PROGRAM = None


def _build_jobs():
    """Job list for one chain: one job per exp-group (<=3 k-tiles of one
    512-q window)."""
    jobs = []
    for di, d in enumerate(DILS):
        Ld = LDS[di]
        bs = min(BLOCK, Ld)
        nblk = Ld // bs
        nkt_blk = bs // 128
        groups = [[0, 1, 2], [3, 4, 5], [6, 7]] if nkt_blk == 8 else [[0, 1], [2, 3]]
        for blk in range(nblk):
            for qc in range(bs // QCH):
                q0 = blk * bs + qc * QCH
                for gi, g in enumerate(groups):
                    jobs.append(
                        dict(
                            di=di,
                            d=d,
                            blk=blk,
                            nkt_blk=nkt_blk,
                            q0=q0,
                            g=g,
                            first=(gi == 0),
                            last=(gi == len(groups) - 1),
                            done0=g[0],
                        )
                    )
    return jobs


def build_program():
    """Build the (SPMD, identical on all cores) Bass program."""
    from contextlib import ExitStack

    import concourse.tile as tile
    from concourse import bacc, mybir

    F32 = mybir.dt.float32
    BF16 = mybir.dt.bfloat16
    EXP_P4, POW8 = _register_exp_ops() if DVE_EXP else (None, None)
    nc = bacc.Bacc("TRN2", target_bir_lowering=False, debug=False)

    blob_d = nc.dram_tensor("blob", [PB, 128, WSUM], BF16, kind="ExternalInput")
    wot_d = nc.dram_tensor("wot", [2, 128, HIDDEN], BF16, kind="ExternalInput")
    out_d = nc.dram_tensor("out", [L, HIDDEN], F32, kind="ExternalOutput")

    with tile.TileContext(nc) as tc, ExitStack() as ctx:
        consts = ctx.enter_context(tc.tile_pool(name="consts", bufs=1))
        br_pool = ctx.enter_context(tc.tile_pool(name="br", bufs=2))
        e_pool = ctx.enter_context(tc.tile_pool(name="ep", bufs=4))
        acc_pool = ctx.enter_context(tc.tile_pool(name="accp", bufs=1))
        io_pool = ctx.enter_context(tc.tile_pool(name="iop", bufs=2))
        st_psum = ctx.enter_context(tc.tile_pool(name="stp", bufs=2, space="PSUM"))
        pv_psum = ctx.enter_context(tc.tile_pool(name="pvp", bufs=2, space="PSUM"))

        zero_bias = consts.tile([128, 1], F32, tag="zb")
        nc.vector.memset(zero_bias, 0.0)
        c4_ap = consts.tile([128, 1], F32, tag="c4")
        nc.vector.memset(c4_ap, EXPC[3])
        # bf16 ones row at partition 64, zeros on 65..127: K=64 zero-padded
        # broadcast weights so the w-broadcast matmul shares the QK matmuls'
        # (64,128) row-tiled PE mode (T8) instead of forcing a mode drain.
        onespad = consts.tile([128, 128], BF16, tag="ones")
        nc.vector.memset(onespad, 0.0)
        nc.vector.memset(onespad[64:65, :], 1.0)
        # staging tiles for the bf16 w row at partition 64; rows 65..127 are
        # zeroed once and never rewritten (only row 64 is written per window)
        wb_tiles = [
            consts.tile([128, QCH], BF16, tag="wb", bufs=2, name=f"wb{i}")
            for i in range(2)
        ]
        for wb in wb_tiles:
            nc.vector.memset(wb[64:128, :], 0.0)
        wb_ctr = [0]

        wot_sb = consts.tile([128, 2, HIDDEN], BF16, tag="wot")
        wot_loaded = [False]

        def load_wot():
            # deferred past the first blob pieces so it doesn't delay the
            # cold-start QK matmuls
            if not wot_loaded[0]:
                wot_loaded[0] = True
                nc.sync.dma_start(
                    out=wot_sb, in_=wot_d.rearrange("j r c -> r j c")
                )

        acc_tiles = [
            acc_pool.tile([128, L], F32, tag=f"acc{j}", bufs=1, name=f"acc{j}")
            for j in range(PB)
        ]

        oacc_pairs = [
            acc_pool.tile([128, L], BF16, tag=f"oacc{p}", bufs=1, name=f"oacc{p}")
            for p in range(2)
        ]
        oacc_tmp = acc_pool.tile([64, L], BF16, tag="otmp", bufs=1, name="oacc_tmp")

        bt_tiles = {}
        dma_issued = set()

        def issue_blob(j, di, v_part=True):
            if j >= PB:
                return
            if (j, di) not in dma_issued:
                dma_issued.add((j, di))
                bufs = 3 if di == 0 else 2
                bt = br_pool.tile(
                    [128, WS[di]], BF16, tag=f"b{di}", bufs=bufs, name=f"bt{di}"
                )
                Ld = LDS[di]
                qk_w = Ld + Ld // 2
                if di == 0 and j < 2:
                    # cold start: land the first window's Q and K columns
                    # first so QK matmuls begin before the full blob arrives
                    pieces = ((0, 1024), (Ld, Ld + 512), (1024, Ld), (Ld + 512, qk_w))
                else:
                    pieces = ((0, qk_w),)
                for c0, c1 in pieces:
                    nc.sync.dma_start(
                        out=bt[:, c0:c1],
                        in_=blob_d[j, :, BOFFS[di] + c0 : BOFFS[di] + c1],
                    )
                bt_tiles[(j, di)] = bt
            if v_part and (j, di, "v") not in dma_issued:
                dma_issued.add((j, di, "v"))
                bt = bt_tiles[(j, di)]
                Ld = LDS[di]
                qk_w = Ld + Ld // 2
                nc.sync.dma_start(
                    out=bt[:, qk_w : WS[di]],
                    in_=blob_d[j, :, BOFFS[di] + qk_w : BOFFS[di] + WS[di]],
                )

        def emit_qk_exp(j, job):
            """QK matmuls for the group -> exp to a bf16 E tile."""
            di, q0, g = job["di"], job["q0"], job["g"]
            Ld = LDS[di]
            bt = bt_tiles[(j, di)]
            gs = len(g)
            st = st_psum.tile([128, 3, QCH], F32, tag="st", name="st")
            for i, kt in enumerate(g):
                tg = job["blk"] * job["nkt_blk"] + kt
                # K parity is flipped on odd pairs (host packs it that way) so
                # the QK stream alternates T0/T8 row tiles across both chains
                # — every matmul overlaps its neighbor on the other row tile.
                half = (tg + j) % 2
                k0 = Ld + (tg // 2) * 128
                nc.tensor.matmul(
                    st[:, i, :],
                    bt[half * 64 : (half + 1) * 64, k0 : k0 + 128],
                    bt[half * 64 : (half + 1) * 64, q0 : q0 + QCH],
                    start=True,
                    stop=True,
                )
            et = e_pool.tile([128, 3, QCH], BF16, tag="et", bufs=5, name="et")
            if job.get("dve"):
                ex = e_pool.tile([128, 3, QCH], F32, tag="ex", bufs=2, name="ex")
                nc.vector._custom_dve(
                    EXP_P4,
                    out=ex[:, 0:gs, :],
                    in0=st[:, 0:gs, :],
                    in1=c4_ap,
                    s0=EXPC[0],
                    s1=EXPC[1],
                    imm2=EXPC[2],
                )
                nc.vector._custom_dve(POW8, out=et[:, 0:gs, :], in0=ex[:, 0:gs, :])
            else:
                nc.scalar.activation(
                    et[:, 0:gs, :],
                    st[:, 0:gs, :],
                    mybir.ActivationFunctionType.Exp,
                    bias=zero_bias,
                    scale=0.125,
                )
            job["et"] = et

        def emit_pv_one(j, job, i):
            di = job["di"]
            Ld = LDS[di]
            vbase = Ld + Ld // 2
            bt = bt_tiles[(j, di)]
            kt = job["g"][i]
            tg = job["blk"] * job["nkt_blk"] + kt
            done = job["done0"] + i
            nc.tensor.matmul(
                job["pv"][0:65, :],
                bt[:, vbase + tg * 65 : vbase + tg * 65 + 65],
                job["et"][:, i, :],
                start=(done == 0),
                stop=(done == job["nkt_blk"] - 1),
                skip_group_check=True,
            )

        def emit_scatter(j, job):
            if not job["last"]:
                return
            d = job["d"]
            acc = acc_tiles[j]
            pv = job["pv"]
            p0 = job["q0"] * d
            if d == 1:
                # NOT on the scalar engine: anything in the ACT FIFO delays
                # the next exp (the phase pacer) — measured +20us
                nc.vector.tensor_copy(out=acc[0:65, p0 : p0 + QCH], in_=pv[0:65, :])
            else:
                dst = acc[0:65, p0 : p0 + QCH * d : d]
                nc.vector.tensor_add(out=dst, in0=dst, in1=pv[0:65, :])

        def emit_pv_pair(job_a, job_b, j_a, j_b):
            """Both chains' PV groups, each chain's accumulation run kept
            contiguous (interleaving the two runs makes walrus break the
            accumulation groups and stalls the PE)."""
            for i in range(len(job_a["g"])):
                emit_pv_one(j_a, job_a, i)
            for i in range(len(job_b["g"])):
                emit_pv_one(j_b, job_b, i)
            emit_scatter(j_a, job_a)
            emit_scatter(j_b, job_b)

        def emit_norm(pp, w, j, tail=False):
            """Normalize one 512-wide window of pair j: stage the bf16 w row
            at partition 64, broadcast it to all partitions with a zero-padded
            K=64 ones-matmul on row-tile T8 (bc rides a spare slice of the st
            PSUM slot), invert on DVE, scale the numerator.  Odd pairs stage
            into oacc_tmp and DMA-stack onto partitions 64..127 for o_proj."""
            ws = slice(w * QCH, (w + 1) * QCH)
            acc = acc_tiles[j]
            wb = wb_tiles[wb_ctr[0] % 2]
            wb_ctr[0] += 1
            if tail:
                nc.scalar.copy(out=wb[64:65, :], in_=acc[64:65, ws])
            else:
                nc.vector.tensor_copy(out=wb[64:65, :], in_=acc[64:65, ws])
            bc = st_psum.tile([128, QCH], F32, tag="st", name="bc")
            nc.tensor.matmul(
                bc,
                onespad[64:128, :],
                wb[64:128, :],
                start=True,
                stop=True,
                tile_position=(64, 0),
            )
            rw = io_pool.tile([64, QCH], F32, tag="rw", name="rw")
            nc.vector.reciprocal_approx_fast(out=rw, in_=bc[0:64, :])
            odd = j % 2 == 1
            dst = oacc_tmp[:, ws] if odd else oacc_pairs[pp][0:64, ws]
            nc.vector.tensor_mul(out=dst, in0=acc[0:64, ws], in1=rw)
            if odd:
                nc.sync.dma_start(
                    out=oacc_pairs[pp][64:128, ws], in_=oacc_tmp[:, ws]
                )

        pending_norm = []
        for pp in range(2):
            ja, jb = 2 * pp, 2 * pp + 1
            if pp == 0:
                # cold start: first-window Q/K columns for both chains, then
                # the first block's V slabs (PV of job 0 needs them), then the
                # remaining Q/K, then the rest of V — so neither engine
                # starves while the 4.4MB initial transfer lands.
                Ld0 = LDS[0]
                qk_w0 = Ld0 + Ld0 // 2
                stage_cols = [
                    (0, 1024),
                    (Ld0, Ld0 + 512),
                    (qk_w0, qk_w0 + 8 * 65),
                    (1024, Ld0),
                    (Ld0 + 512, qk_w0),
                    (qk_w0 + 8 * 65, WS[0]),
                ]
                for j2 in (ja, jb):
                    bt = br_pool.tile(
                        [128, WS[0]], BF16, tag="b0", bufs=3, name="bt0"
                    )
                    bt_tiles[(j2, 0)] = bt
                    dma_issued.add((j2, 0))
                    dma_issued.add((j2, 0, "v"))
                for c0, c1 in stage_cols:
                    for j2 in (ja, jb):
                        nc.sync.dma_start(
                            out=bt_tiles[(j2, 0)][:, c0:c1],
                            in_=blob_d[j2, :, BOFFS[0] + c0 : BOFFS[0] + c1],
                        )
            issue_blob(ja, 0)
            issue_blob(jb, 0)
            issue_blob(ja, 1)
            issue_blob(jb, 1)
            load_wot()
            jobs = {ja: _build_jobs(), jb: _build_jobs()}
            njobs = len(jobs[ja])
            if DVE_EXP:
                # route a staggered ~1/6 of exp groups to the vector engine
                for g in range(njobs - 2):
                    if g % 6 == 1:
                        jobs[ja][g]["dve"] = True
                    elif g % 6 == 4:
                        jobs[jb][g]["dve"] = True
            prev_di = 0
            for g in range(njobs):
                di = jobs[ja][g]["di"]
                if di != prev_di:
                    prev_di = di
                    if di == 1:
                        issue_blob(ja, 2)
                        issue_blob(jb, 2)
                    elif di == 2:
                        issue_blob(ja, 3)
                        issue_blob(jb, 3)
                        issue_blob(ja + 2, 0)
                    elif di == 3:
                        issue_blob(jb + 2, 0)
                        issue_blob(ja + 2, 1)
                        issue_blob(jb + 2, 1)
                # batch both chains' QKs, then both chains' PVs: QK (64-row
                # tiled) and PV (128-row) use different PE tiling modes and
                # each mode change drains the array — 2 switches/iteration
                # instead of 4.
                for j in (ja, jb):
                    job = jobs[j][g]
                    if job["first"]:
                        job["pv"] = pv_psum.tile(
                            [128, QCH], F32, tag="pv", name="pv"
                        )
                    else:
                        job["pv"] = jobs[j][g - 1]["pv"]
                    emit_qk_exp(j, job)
                if g >= 2:
                    emit_pv_pair(jobs[ja][g - 2], jobs[jb][g - 2], ja, jb)
                # drip the previous pair-pair's normalize through this
                # pair-pair's job stream (it touches none of our tiles)
                if jobs[ja][g]["last"]:
                    for _ in range(2):
                        if pending_norm:
                            emit_norm(*pending_norm.pop(0))
            emit_pv_pair(jobs[ja][njobs - 2], jobs[jb][njobs - 2], ja, jb)
            emit_pv_pair(jobs[ja][njobs - 1], jobs[jb][njobs - 1], ja, jb)
            while pending_norm:
                emit_norm(*pending_norm.pop(0))
            pending_norm = [(pp, w, j) for w in range(L // QCH) for j in (ja, jb)]
            for j in (ja, jb):
                del bt_tiles[(j, 0)], bt_tiles[(j, 1)]
                del bt_tiles[(j, 2)], bt_tiles[(j, 3)]

        # tail: last pair-pair's normalize pipelined (one window ahead) with
        # the partial o_proj (out[p, :] = sum_pp oacc_pp[:, p]^T @ wot_pp,
        # K=128 per matmul).  PSUM-evacuation copies alternate between the
        # now-idle scalar engine and the vector engine.
        def emit_oproj(w):
            for mt in range(4 * w, 4 * w + 4):
                # alternate po between the pv slots and the (tail-idle) st
                # slots: 4 banks in flight so the matmul chain never waits on
                # the PSUM-evacuation copy two tiles back
                pool_tag = "pv" if mt % 2 == 0 else "st"
                src = pv_psum if mt % 2 == 0 else st_psum
                po = src.tile([128, HIDDEN], F32, tag=pool_tag, name="po")
                for p in range(2):
                    nc.tensor.matmul(
                        po,
                        oacc_pairs[p][:, mt * 128 : (mt + 1) * 128],
                        wot_sb[:, p, :],
                        start=(p == 0),
                        stop=(p == 1),
                        skip_group_check=True,
                    )
                ot = io_pool.tile([128, HIDDEN], F32, tag="ot", bufs=3)
                if mt % 2 == 0:
                    nc.scalar.copy(out=ot, in_=po)
                else:
                    nc.vector.tensor_copy(out=ot, in_=po)
                nc.sync.dma_start(out=out_d[mt * 128 : (mt + 1) * 128, :], in_=ot)

        # all normalize steps first (they pipeline window-to-window across
        # ACT copy / PE bc / DVE recip+mul), o_proj groups flow behind them
        while pending_norm:
            emit_norm(*pending_norm.pop(0), tail=True)
        for w in range(L // QCH):
            emit_oproj(w)

    nc.compile()
    return nc


def get_program():
    global _PROGRAM
    if _PROGRAM is None:
        _PROGRAM = build_program()
    return _PROGRAM


def _branch_blob(qT, kT, vv, di, flip):
    """Pack one dilation branch into the [128, W] SBUF-layout blob.

    qT, kT: [64, Ld] transposed Q/K for this branch; vv: [Ld, 65] V plus
    ones column.  flip swaps which partition half holds even k-tiles (odd
    pairs are flipped so the interleaved QK stream bricks across PE row
    tiles T0/T8)."""
    Ld, nkt = LDS[di], NKTS[di]
    q_part = np.concatenate([qT, qT], axis=0)  # [128, Ld]
    k3 = kT.reshape(64, nkt, 128)
    halves = [k3[:, 0::2, :].reshape(64, -1), k3[:, 1::2, :].reshape(64, -1)]
    if flip:
        halves = halves[::-1]
    k_part = np.concatenate(halves, axis=0)  # [128, Ld/2]
    v_part = vv.reshape(nkt, 128, 65).transpose(1, 0, 2).reshape(128, nkt * 65)
    return np.concatenate([q_part, k_part, v_part], axis=1)


def make_in_maps(query_states, key_states, value_states, Wo):
    q = np.asarray(query_states, dtype=np.float32)
    k = np.asarray(key_states, dtype=np.float32)
    v = np.asarray(value_states, dtype=np.float32)
    Wo = np.asarray(Wo, dtype=np.float32)

    in_maps = []
    for c in range(NCORES):
        b, hs = c // 2, (c % 2) * PB
        blob = np.empty((PB, 128, WSUM), BF16_NP)
        wot = np.empty((2, 128, HIDDEN), BF16_NP)
        for j in range(PB):
            h = hs + j
            for di, d in enumerate(DILS):
                Ld = LDS[di]
                vv = np.empty((Ld, 65), np.float32)
                vv[:, 0:64] = v[b, h, ::d, :]
                vv[:, 64] = 1.0
                blob[j, :, BOFFS[di] : BOFFS[di] + WS[di]] = _branch_blob(
                    np.ascontiguousarray(q[b, h, ::d, :].T),
                    np.ascontiguousarray(k[b, h, ::d, :].T),
                    vv,
                    di,
                    j % 2 == 1,
                )
        for p in range(2):
            h0 = hs + 2 * p
            wot[p] = Wo[:, h0 * 64 : (h0 + 2) * 64].T
        in_maps.append({"blob": blob, "wot": wot})
    return in_maps


def combine_outputs(results, bo):
    bo = np.asarray(bo, dtype=np.float32)
    out = np.empty((B, L, HIDDEN), np.float32)
    for b in range(B):
        out[b] = results[2 * b]["out"] + results[2 * b + 1]["out"] + bo
    return out


def kernel(
    query_states,
    key_states,
    value_states,
    Wo,
    bo,
    _trace=False,
    _tmpdir=None,
    _results=[None],
):
    from concourse.bass_utils import run_bass_kernel_spmd

    nc = get_program()
    in_maps = make_in_maps(query_states, key_states, value_states, Wo)
    res = run_bass_kernel_spmd(
        nc, in_maps, list(range(NCORES)), trace=_trace, tmpdir=_tmpdir
    )
    _results[0] = res
    return combine_outputs(res.results, bo)
